# revision 18
# baseline (speedup 1.0000x reference)
"""MaxSim (ColBERT) scoring kernel for Trainium2, 8-core SPMD.

Problem: per batch b (1024 total): q[32,128], d[2048,128] f32.
  score[b] = sum_q max_k ( q_hat[q] . d[k] / |d[k]| )
Sharding: batch dim across 8 cores, 128 batches/core. No communication.

The kernel computes in bf16 (meets the 2e-2 gate with ~1e-3 rel err), so
inputs are cast f32->bf16 on the host during staging — halves HBM traffic,
which is the device-side bottleneck.

Per-core plan (bf16 matmul, f32 accumulation):
  - queries: square+rowsum -> |q|^2, 1/sqrt via vector.reciprocal +
    scalar.sqrt, normalize (tensor_scalar), xbar transpose ->
    qT_all[128f, 32*NB] (q_hat columns).
  - docs, per 8-batch load-group on the SP HWDGE ring with 2-group
    prefetch (doc k = 16*p + t -> partition p, tile t; 8KB contiguous
    chunks), one xbar transpose -> docT[128f,128d] tiles.
  - per batch: square+rowsum -> norm2[p, t] split DVE (t<10) / ACT
    (t>=10) so neither engine is the wall; 16 PE matmuls
    simT[128d,32q] = docT.T @ qT_b into one PSUM bank.
  - post stage, software-pipelined one batch behind: inv = 1/sqrt(norm2)
    (ACT sqrt then DVE reciprocal); scaled = simT * inv (tensor_tensor,
    inv broadcast along q with a step-0 AP dim); reduce_max over t; per
    4-batch halfgroup one PE transpose [128,(4b 32q)]->[(4b 32q),128]
    and reduce_max over free -> scores_q2[:, hg].
  - final: scores = ones.T @ scores_q2 (f32 matmul) -> [GB, n_hg] -> HBM.
"""

import os
from contextlib import ExitStack

import ml_dtypes
import numpy as np

import concourse.bass as bass
import concourse.bacc as bacc
import concourse.mybir as mybir
import concourse.tile as tile

F32 = mybir.dt.float32
BF16 = mybir.dt.bfloat16
AX = mybir.AxisListType
OP = mybir.AluOpType
ACT = mybir.ActivationFunctionType

N_CORES = 8
NB_TOTAL = 1024
Q_LEN = 32
D_LEN = 2048
DIM = 128
NB = NB_TOTAL // N_CORES        # 128 batches per core
NT = D_LEN // 128               # 16 doc tiles per batch
GB = 4                          # batches per group (one PSUM-cycle unit)

# Doc-tile square routing: t < T_DVE on DVE, rest on ACT. (GPSIMD would
# help in the cost model, but neuronxcc rejects TensorScalarPtr on Pool,
# so only DVE and ACT can do the square+rowsum.) Weighted by per-tile
# cost (DVE 200ns, ACT 489ns incl. its fixed read-accumulator overhead)
# plus each engine's other duties.
T_DVE = 10


def build_kernel(nc: bass.Bass, tc: tile.TileContext, ctx: ExitStack, nb: int):
    q_dram = nc.dram_tensor("q", [nb, Q_LEN, DIM], BF16, kind="ExternalInput").ap()
    d_dram = nc.dram_tensor("d", [nb, D_LEN, DIM], BF16, kind="ExternalInput").ap()
    identf_dram = nc.dram_tensor("identf", [DIM, DIM], F32, kind="ExternalInput").ap()
    ones_dram = nc.dram_tensor("ones", [GB * Q_LEN, GB], F32, kind="ExternalInput").ap()
    out_dram = nc.dram_tensor("scores", [1, nb], F32, kind="ExternalOutput").ap()

    nqt = (nb * Q_LEN) // 128   # query prep tiles (4 batches each)

    const_pool = ctx.enter_context(tc.tile_pool(name="const", bufs=1))
    qprep_pool = ctx.enter_context(tc.tile_pool(name="qprep", bufs=1))
    qT_pool = ctx.enter_context(tc.tile_pool(name="qT", bufs=1))
    dnat_pool = ctx.enter_context(tc.tile_pool(name="dnat", bufs=3))
    docT_pool = ctx.enter_context(tc.tile_pool(name="docT", bufs=2))
    sq_pool = ctx.enter_context(tc.tile_pool(name="sqjunk", bufs=6))
    norm_pool = ctx.enter_context(tc.tile_pool(name="norm", bufs=3))
    scaled_pool = ctx.enter_context(tc.tile_pool(name="scaled", bufs=4))
    maxs_pool = ctx.enter_context(tc.tile_pool(name="maxs", bufs=2))
    scoresq_pool = ctx.enter_context(tc.tile_pool(name="scoresq", bufs=1))
    srow_pool = ctx.enter_context(tc.tile_pool(name="srow", bufs=1))

    psum_sim = ctx.enter_context(tc.tile_pool(name="psim", bufs=5, space="PSUM"))
    psum_tr = ctx.enter_context(tc.tile_pool(name="ptr", bufs=2, space="PSUM"))
    psum_fin = ctx.enter_context(tc.tile_pool(name="pfin", bufs=1, space="PSUM"))

    # ---- constants ----
    identf = const_pool.tile([DIM, DIM], F32, tag="identf")
    nc.sync.dma_start(identf[:], identf_dram)
    ones = const_pool.tile([GB * Q_LEN, GB], F32, tag="ones")
    nc.sync.dma_start(ones[:], ones_dram)

    # ---- d-load pipeline (hoisted ahead of q-prep) ----
    # The first two 8-batch d-loads go on the SP ring before q-prep's
    # xbar transpose, which waits on q-prep compute: otherwise that wait
    # blocks the ring head and delays the whole d pipeline by ~15us.
    LB = 2 * GB                  # batches per load-group
    n_lg = nb // LB
    dnat_tiles = {}

    def issue_load(g):
        d_nat = dnat_pool.tile([128, LB, NT, DIM], BF16, tag="dnat")
        dnat_tiles[g] = d_nat
        src = d_dram[g * LB : (g + 1) * LB].rearrange(
            "b (p t) f -> p b (t f)", p=128
        )
        nc.sync.dma_start(d_nat.rearrange("p b t f -> p b (t f)"), src)

    for g in range(min(2, n_lg)):
        issue_load(g)

    # ---- query prep ----
    # q_flat[(nb*32), 128]; tile g covers rows 128g..128g+127 (4 batches).
    q_rows = q_dram.rearrange("b q f -> (b q) f")
    q_nat = qprep_pool.tile([128, nqt, DIM], BF16, tag="qnat")
    nc.sync.dma_start(q_nat[:], q_rows.rearrange("(g p) f -> p g f", p=128))
    qn2 = qprep_pool.tile([128, nqt], F32, tag="qn2")
    qinv = qprep_pool.tile([128, nqt], F32, tag="qinv")
    qT_all = qT_pool.tile([DIM, nb * Q_LEN], BF16, tag="qTall")
    for g in range(nqt):
        nc.vector.scalar_tensor_tensor(
            out=sq_pool.tile([128, DIM], BF16, tag="sqq", name="sqj_q"),
            in0=q_nat[:, g],
            scalar=1.0,
            in1=q_nat[:, g],
            op0=OP.mult,
            op1=OP.mult,
            accum_out=qn2[:, g : g + 1],
        )
    nc.vector.reciprocal(qinv[:], qn2[:])
    nc.scalar.sqrt(qinv[:], qinv[:])  # qinv = 1/|q|
    qnrm = qprep_pool.tile([128, nqt, DIM], BF16, tag="qnrm")
    for g in range(nqt):
        nc.vector.tensor_scalar(
            out=qnrm[:, g],
            in0=q_nat[:, g],
            scalar1=qinv[:, g : g + 1],
            scalar2=None,
            op0=OP.mult,
        )
    # one xbar transpose: qT_all[:, g, j] = qnrm[j, g, :]
    # wait-absorber spanning every qnrm write (one elem per g)
    nc.sync.dma_start(qT_all[0:1, 0 : 2 * nqt], qnrm[0:1, :, 0:2])
    nc.sync.dma_start_transpose(
        qT_all[:].rearrange("f (g j) -> f g j", g=nqt),
        qnrm.rearrange("p g f -> p (g f)"),
    )

    # ---- main loop over doc batches ----
    # DMA in 8-batch load-groups (halves the per-DMA fixed-latency bubble
    # count); compute + PSUM in 4-batch halfgroups (the maxs transpose is
    # capped at 128 output partitions = 4 batches x 32 q).
    # Ring order [load0][load1][xpose0][load2][xpose1]... keeps every ring
    # entry's RAW dependency >=1 full entry back, so the SP sequencer's
    # wait-then-push never idles the DMA engines.
    # Post-matmul work for batch X (rsqrt, scale, reduce) is issued after
    # batch X+1's squares+matmuls so its cross-engine waits (ACT sqrt
    # needs all three square engines; the DVE scale needs the sqrt)
    # resolve with a full batch of slack.
    n_hg = nb // GB              # halfgroups = score columns
    scores_q2 = scoresq_pool.tile([GB * Q_LEN, n_hg], F32, tag="scoresq")
    pending = []  # queued (hg, bi4, bank, norm2, bi8, maxs4) post work

    def do_post(hg, bi4, bank, norm2, bi8, maxs4):
        # inv = 1/|d|: sqrt on ACT first (waits on the square engines),
        # then reciprocal on DVE feeding the DVE scale directly.
        sl = norm2[:, bi8]
        nc.scalar.sqrt(sl, sl)
        nc.vector.reciprocal(sl, sl)
        # scaled[p, q, t] = bank[p, t, q] * inv[p, bi8, t]
        scaled = scaled_pool.tile([128, Q_LEN, NT], BF16, tag="scaled",
                                  name="scaled")
        bank_qt = bank[:].rearrange("p (t q) -> p q t", t=NT)
        inv_bi = norm2[:, bi8]
        inv_b = bass.AP(
            inv_bi.tensor,
            inv_bi.offset,
            [inv_bi.ap[0], [0, Q_LEN], inv_bi.ap[1]],
        )
        nc.vector.tensor_tensor(
            out=scaled[:], in0=bank_qt, in1=inv_b, op=OP.mult
        )
        # max over t (the 16 docs sharing each partition)
        nc.vector.tensor_reduce(
            out=maxs4[:, bi8], in_=scaled[:], axis=AX.X, op=OP.max
        )
        if bi4 == GB - 1:
            # cross-partition max for the halfgroup: transpose
            # [128, GB*32] -> [(bi q), p] then reduce over p
            h0 = (bi8 - GB + 1)
            tr = psum_tr.tile([GB * Q_LEN, 128], F32, tag="tr")
            nc.tensor.transpose(
                tr[:],
                maxs4[:, h0 : h0 + GB].rearrange("p b q -> p (b q)"),
                identf[:],
            )
            nc.vector.tensor_reduce(
                out=scores_q2[:, hg : hg + 1], in_=tr[:], axis=AX.X, op=OP.max
            )

    for lg in range(n_lg):
        d_nat = dnat_tiles.pop(lg)

        # one xbar transpose per load-group
        docT_all = docT_pool.tile([DIM, LB * NT, 128], BF16, tag="docT")
        nc.sync.dma_start_transpose(
            docT_all[:], d_nat.rearrange("p b t f -> p (b t f)")
        )
        if lg + 2 < n_lg:
            issue_load(lg + 2)

        norm2 = norm_pool.tile([128, LB, NT], F32, tag="norm2")
        maxs4 = maxs_pool.tile([128, LB, Q_LEN], F32, tag="maxs")
        for bi8 in range(LB):
            b = lg * LB + bi8
            hg = b // GB
            bi4 = b % GB
            bank = psum_sim.tile([128, NT * Q_LEN], F32, tag="bank", name="bank")
            t_dve = T_DVE  # flat split: hedges model (187ns) vs real (~279ns) ACT read-accum cost
            for t in range(NT):
                dt_tile = d_nat[:, bi8, t]
                # norms: square + row-sum; split DVE / ACT so the
                # elementwise pass isn't one engine's wall.
                if t < t_dve:
                    nc.vector.scalar_tensor_tensor(
                        out=sq_pool.tile([128, DIM], BF16, tag="sqd", name="sqj_d"),
                        in0=dt_tile,
                        scalar=1.0,
                        in1=dt_tile,
                        op0=OP.mult,
                        op1=OP.mult,
                        accum_out=norm2[:, bi8, t : t + 1],
                    )
                else:
                    nc.scalar.activation(
                        out=sq_pool.tile([128, DIM], BF16, tag="sqa", name="sqa_d"),
                        in_=dt_tile,
                        func=ACT.Square,
                        accum_out=norm2[:, bi8, t : t + 1],
                    )
                # simT[128d, 32q] into bank columns t*32..t*32+32
                nc.tensor.matmul(
                    bank[:, t * Q_LEN : (t + 1) * Q_LEN],
                    lhsT=docT_all[:, bi8 * NT + t, :],
                    rhs=qT_all[:, b * Q_LEN : (b + 1) * Q_LEN],
                    start=True,
                    stop=True,
                )
            pending.append((hg, bi4, bank, norm2, bi8, maxs4))
            if len(pending) > 1:
                do_post(*pending.pop(0))
    while pending:
        do_post(*pending.pop(0))

    # ---- final: fin[bi, g] = sum_q scores_q2[bi*32+q, g] = score[GB*g+bi]
    fin = psum_fin.tile([GB, n_hg], F32, tag="fin")
    nc.tensor.matmul(
        fin[:], lhsT=ones[:], rhs=scores_q2[:], start=True, stop=True
    )
    srow = srow_pool.tile([GB, n_hg], F32, tag="srow")
    nc.scalar.copy(srow[:], fin[:])
    nc.sync.dma_start(
        out_dram.rearrange("o (g bi) -> (o bi) g", bi=GB), srow[:]
    )


def _build(nb: int) -> bass.Bass:
    nc = bacc.Bacc("TRN2", target_bir_lowering=False, debug=False)
    with tile.TileContext(nc) as tc:
        with ExitStack() as ctx:
            build_kernel(nc, tc, ctx, nb)
    nc.compile()
    return nc


def _consts() -> dict[str, np.ndarray]:
    return {
        "identf": np.eye(DIM, dtype=np.float32),
        "ones": np.kron(np.eye(GB, dtype=np.float32), np.ones((Q_LEN, 1), np.float32)),
    }


def _prep_in_maps(q: np.ndarray, d: np.ndarray) -> list[dict[str, np.ndarray]]:
    """Per-core input staging: slice the batch dim and cast to bf16 (the
    kernel's compute dtype) so the device only ever sees bf16 bytes."""
    qh = np.ascontiguousarray(q).astype(ml_dtypes.bfloat16)
    dh = np.ascontiguousarray(d).astype(ml_dtypes.bfloat16)
    consts = _consts()
    in_maps = []
    for c in range(N_CORES):
        sl = slice(c * NB, (c + 1) * NB)
        in_maps.append({"q": qh[sl], "d": dh[sl], **consts})
    return in_maps


def kernel(**inputs: np.ndarray) -> np.ndarray:
    from concourse import bass_utils

    q = np.asarray(inputs["query_embeddings"], dtype=np.float32)
    d = np.asarray(inputs["doc_embeddings"], dtype=np.float32)
    assert q.shape == (NB_TOTAL, Q_LEN, DIM) and d.shape == (NB_TOTAL, D_LEN, DIM)

    nc = _build(NB)
    in_maps = _prep_in_maps(q, d)
    res = bass_utils.run_bass_kernel_spmd(
        nc,
        in_maps,
        core_ids=list(range(N_CORES)),
        trace=bool(int(os.environ.get("MAXSIM_TRACE", "0"))),
    )
    out = np.concatenate(
        [res.results[c]["scores"].reshape(-1) for c in range(N_CORES)]
    ).astype(np.float32)
    return out


# revision 19
# speedup vs baseline: 1.0811x; 1.0811x over previous
"""MaxSim (ColBERT) scoring kernel for Trainium2, 8-core SPMD.

Problem: per batch b (1024 total): q[32,128], d[2048,128] f32.
  score[b] = sum_q max_k ( q_hat[q] . d[k] / |d[k]| )
Sharding: batch dim across 8 cores, 128 batches/core. No communication.

The kernel computes in bf16 (meets the 2e-2 gate with ~1e-3 rel err), so
inputs are cast f32->bf16 on the host during staging — halves HBM traffic,
which is the device-side bottleneck.

Per-core plan (bf16 matmul, f32 accumulation):
  - queries: square+rowsum -> |q|^2, 1/sqrt via vector.reciprocal +
    scalar.sqrt, normalize (tensor_scalar), xbar transpose ->
    qT_all[128f, 32*NB] (q_hat columns).
  - docs, per 8-batch load-group on the SP HWDGE ring with 2-group
    prefetch (doc k = 16*p + t -> partition p, tile t; 8KB contiguous
    chunks), one xbar transpose -> docT[128f,128d] tiles.
  - per batch: square+rowsum -> norm2[p, t] split DVE (t<10) / ACT
    (t>=10) so neither engine is the wall; 16 PE matmuls
    simT[128d,32q] = docT.T @ qT_b into one PSUM bank.
  - post stage, software-pipelined one batch behind: inv = 1/sqrt(norm2)
    (ACT sqrt then DVE reciprocal); scaled = simT * inv (tensor_tensor,
    inv broadcast along q with a step-0 AP dim); reduce_max over t; per
    4-batch halfgroup one PE transpose [128,(4b 32q)]->[(4b 32q),128]
    and reduce_max over free -> scores_q2[:, hg].
  - final: scores = ones.T @ scores_q2 (f32 matmul) -> [GB, n_hg] -> HBM.
"""

import os
from contextlib import ExitStack

import ml_dtypes
import numpy as np

import concourse.bass as bass
import concourse.bacc as bacc
import concourse.mybir as mybir
import concourse.tile as tile

F32 = mybir.dt.float32
BF16 = mybir.dt.bfloat16
AX = mybir.AxisListType
OP = mybir.AluOpType
ACT = mybir.ActivationFunctionType

N_CORES = 8
NB_TOTAL = 1024
Q_LEN = 32
D_LEN = 2048
DIM = 128
NB = NB_TOTAL // N_CORES        # 128 batches per core
NT = D_LEN // 128               # 16 doc tiles per batch
GB = 4                          # batches per group (one PSUM-cycle unit)

# Doc-tile square routing: t < T_DVE on DVE, rest on ACT. (GPSIMD would
# help in the cost model, but neuronxcc rejects TensorScalarPtr on Pool,
# so only DVE and ACT can do the square+rowsum.) Weighted by per-tile
# cost (DVE 200ns, ACT 489ns incl. its fixed read-accumulator overhead)
# plus each engine's other duties.
T_DVE = 10


def build_kernel(nc: bass.Bass, tc: tile.TileContext, ctx: ExitStack, nb: int):
    q_dram = nc.dram_tensor("q", [nb, Q_LEN, DIM], BF16, kind="ExternalInput").ap()
    d_dram = nc.dram_tensor("d", [nb, D_LEN, DIM], BF16, kind="ExternalInput").ap()
    identf_dram = nc.dram_tensor("identf", [DIM, DIM], F32, kind="ExternalInput").ap()
    ones_dram = nc.dram_tensor("ones", [GB * Q_LEN, GB], F32, kind="ExternalInput").ap()
    out_dram = nc.dram_tensor("scores", [1, nb], F32, kind="ExternalOutput").ap()

    nqt = (nb * Q_LEN) // 128   # query prep tiles (4 batches each)

    const_pool = ctx.enter_context(tc.tile_pool(name="const", bufs=1))
    qprep_pool = ctx.enter_context(tc.tile_pool(name="qprep", bufs=1))
    qT_pool = ctx.enter_context(tc.tile_pool(name="qT", bufs=1))
    dnat_pool = ctx.enter_context(tc.tile_pool(name="dnat", bufs=3))
    docT_pool = ctx.enter_context(tc.tile_pool(name="docT", bufs=2))
    sq_pool = ctx.enter_context(tc.tile_pool(name="sqjunk", bufs=6))
    norm_pool = ctx.enter_context(tc.tile_pool(name="norm", bufs=3))
    scaled_pool = ctx.enter_context(tc.tile_pool(name="scaled", bufs=4))
    maxs_pool = ctx.enter_context(tc.tile_pool(name="maxs", bufs=2))
    scoresq_pool = ctx.enter_context(tc.tile_pool(name="scoresq", bufs=1))
    srow_pool = ctx.enter_context(tc.tile_pool(name="srow", bufs=1))

    psum_sim = ctx.enter_context(tc.tile_pool(name="psim", bufs=5, space="PSUM"))
    psum_tr = ctx.enter_context(tc.tile_pool(name="ptr", bufs=2, space="PSUM"))
    psum_fin = ctx.enter_context(tc.tile_pool(name="pfin", bufs=1, space="PSUM"))

    # ---- constants ----
    identf = const_pool.tile([DIM, DIM], F32, tag="identf")
    nc.sync.dma_start(identf[:], identf_dram)
    ones = const_pool.tile([GB * Q_LEN, GB], F32, tag="ones")
    nc.sync.dma_start(ones[:], ones_dram)

    # ---- d-load pipeline (hoisted ahead of q-prep) ----
    # The first two 8-batch d-loads go on the SP ring before q-prep's
    # xbar transpose, which waits on q-prep compute: otherwise that wait
    # blocks the ring head and delays the whole d pipeline by ~15us.
    LB = 2 * GB                  # batches per load-group
    n_lg = nb // LB
    dnat_tiles = {}

    def issue_load(g):
        d_nat = dnat_pool.tile([128, LB, NT, DIM], BF16, tag="dnat")
        dnat_tiles[g] = d_nat
        src = d_dram[g * LB : (g + 1) * LB].rearrange(
            "b (p t) f -> p b (t f)", p=128
        )
        nc.sync.dma_start(d_nat.rearrange("p b t f -> p b (t f)"), src)

    for g in range(min(2, n_lg)):
        issue_load(g)

    # ---- query prep ----
    # q_flat[(nb*32), 128]; tile g covers rows 128g..128g+127 (4 batches).
    q_rows = q_dram.rearrange("b q f -> (b q) f")
    q_nat = qprep_pool.tile([128, nqt, DIM], BF16, tag="qnat")
    nc.sync.dma_start(q_nat[:], q_rows.rearrange("(g p) f -> p g f", p=128))
    qn2 = qprep_pool.tile([128, nqt], F32, tag="qn2")
    qinv = qprep_pool.tile([128, nqt], F32, tag="qinv")
    qT_all = qT_pool.tile([DIM, nb * Q_LEN], BF16, tag="qTall")
    for g in range(nqt):
        nc.vector.scalar_tensor_tensor(
            out=sq_pool.tile([128, DIM], BF16, tag="sqq", name="sqj_q"),
            in0=q_nat[:, g],
            scalar=1.0,
            in1=q_nat[:, g],
            op0=OP.mult,
            op1=OP.mult,
            accum_out=qn2[:, g : g + 1],
        )
    nc.vector.reciprocal(qinv[:], qn2[:])
    nc.scalar.sqrt(qinv[:], qinv[:])  # qinv = 1/|q|
    qnrm = qprep_pool.tile([128, nqt, DIM], BF16, tag="qnrm")
    for g in range(nqt):
        nc.vector.tensor_scalar(
            out=qnrm[:, g],
            in0=q_nat[:, g],
            scalar1=qinv[:, g : g + 1],
            scalar2=None,
            op0=OP.mult,
        )
    # one xbar transpose: qT_all[:, g, j] = qnrm[j, g, :]
    # wait-absorber spanning every qnrm write (one elem per g)
    nc.sync.dma_start(qT_all[0:1, 0 : 2 * nqt], qnrm[0:1, :, 0:2])
    nc.sync.dma_start_transpose(
        qT_all[:].rearrange("f (g j) -> f g j", g=nqt),
        qnrm.rearrange("p g f -> p (g f)"),
    )

    # ---- main loop over doc batches ----
    # DMA in 8-batch load-groups (halves the per-DMA fixed-latency bubble
    # count); compute + PSUM in 4-batch halfgroups (the maxs transpose is
    # capped at 128 output partitions = 4 batches x 32 q).
    # Ring order [load0][load1][xpose0][load2][xpose1]... keeps every ring
    # entry's RAW dependency >=1 full entry back, so the SP sequencer's
    # wait-then-push never idles the DMA engines.
    # Post-matmul work for batch X (rsqrt, scale, reduce) is issued after
    # batch X+1's squares+matmuls so its cross-engine waits (ACT sqrt
    # needs all three square engines; the DVE scale needs the sqrt)
    # resolve with a full batch of slack.
    n_hg = nb // GB              # halfgroups = score columns
    scores_q2 = scoresq_pool.tile([GB * Q_LEN, n_hg], F32, tag="scoresq")
    pending = []  # queued (hg, bi4, bank, norm2, bi8, maxs4) post work

    def do_post(hg, bi4, bank, norm2, bi8, maxs4):
        # inv = 1/|d|: sqrt on ACT first (waits on the square engines),
        # then reciprocal on DVE feeding the DVE scale directly.
        sl = norm2[:, bi8]
        nc.scalar.sqrt(sl, sl)
        nc.vector.reciprocal(sl, sl)
        # scaled[p, q, t] = bank[p, t, q] * inv[p, bi8, t]
        scaled = scaled_pool.tile([128, Q_LEN, NT], BF16, tag="scaled",
                                  name="scaled")
        bank_qt = bank[:].rearrange("p (t q) -> p q t", t=NT)
        inv_bi = norm2[:, bi8]
        inv_b = bass.AP(
            inv_bi.tensor,
            inv_bi.offset,
            [inv_bi.ap[0], [0, Q_LEN], inv_bi.ap[1]],
        )
        nc.vector.tensor_tensor(
            out=scaled[:], in0=bank_qt, in1=inv_b, op=OP.mult
        )
        # max over t (the 16 docs sharing each partition)
        nc.vector.tensor_reduce(
            out=maxs4[:, bi8], in_=scaled[:], axis=AX.X, op=OP.max
        )
        if bi4 == GB - 1:
            # cross-partition max for the halfgroup: transpose
            # [128, GB*32] -> [(bi q), p] then reduce over p
            h0 = (bi8 - GB + 1)
            tr = psum_tr.tile([GB * Q_LEN, 128], F32, tag="tr")
            nc.tensor.transpose(
                tr[:],
                maxs4[:, h0 : h0 + GB].rearrange("p b q -> p (b q)"),
                identf[:],
            )
            nc.vector.tensor_reduce(
                out=scores_q2[:, hg : hg + 1], in_=tr[:], axis=AX.X, op=OP.max
            )

    for lg in range(n_lg):
        d_nat = dnat_tiles.pop(lg)

        # one xbar transpose per load-group
        docT_all = docT_pool.tile([DIM, LB * NT, 128], BF16, tag="docT")
        nc.sync.dma_start_transpose(
            docT_all[:], d_nat.rearrange("p b t f -> p (b t f)")
        )
        if lg + 2 < n_lg:
            issue_load(lg + 2)

        norm2 = norm_pool.tile([128, LB, NT], F32, tag="norm2")
        maxs4 = maxs_pool.tile([128, LB, Q_LEN], F32, tag="maxs")
        for bi8 in range(LB):
            b = lg * LB + bi8
            hg = b // GB
            bi4 = b % GB
            bank = psum_sim.tile([128, NT * Q_LEN], F32, tag="bank", name="bank")
            t_dve = T_DVE - (bi8 % 2)  # alternate 10/9, averages 9.5
            for t in range(NT):
                dt_tile = d_nat[:, bi8, t]
                # norms: square + row-sum; split DVE / ACT so the
                # elementwise pass isn't one engine's wall.
                if t < t_dve:
                    nc.vector.scalar_tensor_tensor(
                        out=sq_pool.tile([128, DIM], BF16, tag="sqd", name="sqj_d"),
                        in0=dt_tile,
                        scalar=1.0,
                        in1=dt_tile,
                        op0=OP.mult,
                        op1=OP.mult,
                        accum_out=norm2[:, bi8, t : t + 1],
                    )
                else:
                    nc.scalar.activation(
                        out=sq_pool.tile([128, DIM], BF16, tag="sqa", name="sqa_d"),
                        in_=dt_tile,
                        func=ACT.Square,
                        accum_out=norm2[:, bi8, t : t + 1],
                    )
                # simT[128d, 32q] into bank columns t*32..t*32+32
                nc.tensor.matmul(
                    bank[:, t * Q_LEN : (t + 1) * Q_LEN],
                    lhsT=docT_all[:, bi8 * NT + t, :],
                    rhs=qT_all[:, b * Q_LEN : (b + 1) * Q_LEN],
                    start=True,
                    stop=True,
                )
            pending.append((hg, bi4, bank, norm2, bi8, maxs4))
            if len(pending) > 1:
                do_post(*pending.pop(0))
    while pending:
        do_post(*pending.pop(0))

    # ---- final: fin[bi, g] = sum_q scores_q2[bi*32+q, g] = score[GB*g+bi]
    fin = psum_fin.tile([GB, n_hg], F32, tag="fin")
    nc.tensor.matmul(
        fin[:], lhsT=ones[:], rhs=scores_q2[:], start=True, stop=True
    )
    srow = srow_pool.tile([GB, n_hg], F32, tag="srow")
    nc.scalar.copy(srow[:], fin[:])
    nc.sync.dma_start(
        out_dram.rearrange("o (g bi) -> (o bi) g", bi=GB), srow[:]
    )


def _build(nb: int) -> bass.Bass:
    nc = bacc.Bacc("TRN2", target_bir_lowering=False, debug=False)
    with tile.TileContext(nc) as tc:
        with ExitStack() as ctx:
            build_kernel(nc, tc, ctx, nb)
    nc.compile()
    return nc


def _consts() -> dict[str, np.ndarray]:
    return {
        "identf": np.eye(DIM, dtype=np.float32),
        "ones": np.kron(np.eye(GB, dtype=np.float32), np.ones((Q_LEN, 1), np.float32)),
    }


def _prep_in_maps(q: np.ndarray, d: np.ndarray) -> list[dict[str, np.ndarray]]:
    """Per-core input staging: slice the batch dim and cast to bf16 (the
    kernel's compute dtype) so the device only ever sees bf16 bytes."""
    qh = np.ascontiguousarray(q).astype(ml_dtypes.bfloat16)
    dh = np.ascontiguousarray(d).astype(ml_dtypes.bfloat16)
    consts = _consts()
    in_maps = []
    for c in range(N_CORES):
        sl = slice(c * NB, (c + 1) * NB)
        in_maps.append({"q": qh[sl], "d": dh[sl], **consts})
    return in_maps


def kernel(**inputs: np.ndarray) -> np.ndarray:
    from concourse import bass_utils

    q = np.asarray(inputs["query_embeddings"], dtype=np.float32)
    d = np.asarray(inputs["doc_embeddings"], dtype=np.float32)
    assert q.shape == (NB_TOTAL, Q_LEN, DIM) and d.shape == (NB_TOTAL, D_LEN, DIM)

    nc = _build(NB)
    in_maps = _prep_in_maps(q, d)
    res = bass_utils.run_bass_kernel_spmd(
        nc,
        in_maps,
        core_ids=list(range(N_CORES)),
        trace=bool(int(os.environ.get("MAXSIM_TRACE", "0"))),
    )
    out = np.concatenate(
        [res.results[c]["scores"].reshape(-1) for c in range(N_CORES)]
    ).astype(np.float32)
    return out
